# revision 28
# baseline (speedup 1.0000x reference)
"""TopK sparse autoencoder forward pass on 8 TRN2 NeuronCores.

Strategy: data-parallel over the token batch (8192 rows -> 1024 rows/core,
zero collectives). Per core, the batch is processed in two halves so the
second half's encode overlaps the first half's sparse decode:
  1. encode: pre = x @ W_enc.T + b_eff as fp32r matmuls at full PE rate
     (batch rows on partitions, features on the free dim); the x - b_dec
     subtraction is folded into b_eff host-side. ReLU fused into the
     PSUM->SBUF copy on the Activation engine.
  2. top-64 with indices: per 256-wide feature group, DVE Max8 + MaxIndex
     give the top-8 values and their within-group positions; the position is
     packed into the low 8 mantissa bits of the value (2^-15 relative
     perturbation, far below fp32r noise). 8 rounds of Max8 + MaxIndex +
     MatchReplace over the 1152 packed candidates yield the top-64 packed
     values and their candidate positions; the feature index is recovered
     arithmetically: feat = 256*(pos>>3) | (bits(val) & 0xFF).
  3. sparse decode: for each of the 64 slots, an indirect DMA gathers
     W_dec^T rows (bf16) by feature index, one row per token partition.
     Half A (overlapped with half B's encode): Act scales G*val -> fp16,
     Pool accumulates — DVE stays free for half B's candidate scan, and
     the ops are emission-interleaved into the encode loop to avoid
     head-of-line blocking on the in-order engine queues. The final half
     accumulates with DVE scalar_tensor_tensor (Pool is busy generating
     gather descriptors).
"""

import os
import numpy as np
import ml_dtypes

from concourse import bass, mybir
from concourse import tile
from concourse.bass_utils import run_bass_kernel_spmd

F32 = mybir.dt.float32
F32R = mybir.dt.float32r
F16 = mybir.dt.float16
U32 = mybir.dt.uint32
U16 = mybir.dt.uint16
BF16 = mybir.dt.bfloat16

N_CORES = 8
B, D, F, K = 8192, 2304, 36864, 64

PT = 128           # partition tile
FT = 512           # encode feature tile (matmul moving dim)
GRP = 512          # max8 candidate group width (9-bit in-group position;
                   # P(a group holds >8 of a token's top-64) ~ 3e-5/token)
GBITS = (GRP - 1).bit_length()          # bits for the in-group position
VMASK = (~((1 << GBITS) - 1)) & 0xFFFFFFFF
LMASK = (1 << GBITS) - 1
N_HALF = 1         # batch halves for encode/decode overlap (1 = no split:
                   # re-streaming W_enc for a second half makes that half's
                   # encode DMA-bound and costs more than the overlap saves)


def split_waits(nc, maxw=1):
    """Walrus in this container accepts few sync-waits per instruction; Tile
    emits many. Move excess waits onto standalone same-engine no-ops."""
    for fn in nc.m.functions:
        for blk in fn.blocks:
            newinsts = []
            for inst in blk.instructions:
                si = inst.sync_info
                if si is not None and len(si.on_wait) > maxw:
                    extra = si.on_wait[:-maxw]
                    keep = si.on_wait[-maxw:]
                    for j, w in enumerate(extra):
                        nop = mybir.InstNoOp(name=f"{inst.name}-wsplit{j}", ins=[], outs=[])
                        nop.engine = inst.engine
                        nop.sync_info = mybir.SyncInfo(on_wait=[w], on_update=[])
                        newinsts.append(nop)
                    si.on_wait = keep
                newinsts.append(inst)
            blk.instructions = newinsts


def build_nc(b_loc, d, f, with_bias, with_bdec):
    nbt = b_loc // PT          # batch tiles
    nd = d // PT               # contraction chunks
    nft = f // FT              # encode feature tiles
    ngrp = f // GRP            # candidate groups per row
    ncand = ngrp * 8           # candidates per row
    assert ncand >= K and K % 8 == 0
    nrounds = K // 8
    assert nbt % N_HALF == 0
    nbh = nbt // N_HALF        # batch tiles per half

    nc = bass.Bass()
    # x and weights declared float32r: same 4-byte container as f32, PE
    # rounds internally — lets plain DMAs satisfy the BIR fp32r-producer
    # rule with zero cast instructions.
    xT = nc.declare_dram_parameter("xT", [d, b_loc], F32R, isOutput=False)
    wencT = nc.declare_dram_parameter("W_encT", [d, f], F32R, isOutput=False)
    wdec16 = nc.declare_dram_parameter("Wdec16", [f, d], BF16, isOutput=False)
    if with_bias:
        b_enc = nc.declare_dram_parameter("b_enc", [f], F32R, isOutput=False)
    if with_bdec:
        bdecb = nc.declare_dram_parameter("bdecb", [PT, d], F32, isOutput=False)
    out = nc.declare_dram_parameter("out", [b_loc, d], F32, isOutput=True)

    wencT_r = wencT.rearrange("(a p) f -> p a f", p=PT)   # [128, nd, f]
    xT_r = xT.rearrange("(a p) b -> p a b", p=PT)         # [128, nd, b_loc]
    out_r = out.rearrange("(t p) e -> t p e", p=PT)       # [nbt, 128, d]

    with tile.TileContext(nc) as tc:
        with tc.tile_pool(name="persist", bufs=1) as pp:
            if with_bias:
                ones_st = pp.tile([1, PT], F32)
                nc.vector.memset(ones_st[:, :], 1.0)
                ones = pp.tile([1, PT], F32R)
                nc.vector.tensor_copy(ones[:, :], ones_st[:, :])
            if with_bdec:
                bdecb_sb = pp.tile([PT, d], F32)
                nc.sync.dma_start(out=bdecb_sb[:, :], in_=bdecb[:, :])

            # x tiles, resident for the whole encode
            xs = []
            for a in range(nd):
                xt = pp.tile([PT, b_loc], F32R, name=f"xs{a}")
                nc.sync.dma_start(out=xt[:, :], in_=xT_r[:, a, :])
                xs.append(xt)

            with tc.tile_pool(name="candV", bufs=nbh) as cvp, \
                 tc.tile_pool(name="candR", bufs=nbh) as crp, \
                 tc.tile_pool(name="tpf", bufs=8) as tpf, \
                 tc.tile_pool(name="enc_w", bufs=nd + 2) as wp, \
                 tc.tile_pool(name="enc_b", bufs=2) as bp, \
                 tc.tile_pool(name="enc_ast", bufs=3) as ap_, \
                 tc.tile_pool(name="psum_e", bufs=4, space="PSUM") as pse, \
                 tc.tile_pool(name="dec_g", bufs=3) as gp, \
                 tc.tile_pool(name="dec_sg", bufs=2) as sgp, \
                 tc.tile_pool(name="dec_acc", bufs=2) as accp:

                def encode_half(h, candV, candR, interleave=None):
                    """Emit encode+candidate ops for half h; pull from the
                    `interleave` generator (previous half's decode) after
                    each feature tile to keep the in-order engine queues
                    from head-blocking."""
                    bts = list(range(h * nbh, (h + 1) * nbh))
                    for ft in range(nft):
                        f0 = ft * FT
                        ws = []
                        for a in range(nd):
                            wst = wp.tile([PT, FT], F32R, tag="wst", name=f"wst{h}_{ft}_{a}")
                            nc.sync.dma_start(
                                out=wst[:, :], in_=wencT_r[:, a, f0 : f0 + FT]
                            )
                            ws.append(wst)
                        if with_bias:
                            bes = bp.tile([1, FT], F32R, tag="bes", name=f"bes{h}_{ft}")
                            nc.sync.dma_start(
                                out=bes[:, :],
                                in_=b_enc.rearrange("(o x) -> o x", o=1)[:, f0 : f0 + FT],
                            )
                        for bt in bts:
                            ps = pse.tile([PT, FT], F32, tag="pse", name=f"pse{h}_{ft}_{bt}")
                            for a in range(nd):
                                nc.tensor.matmul(
                                    ps[:, :],
                                    lhsT=xs[a][:, bt * PT : (bt + 1) * PT],
                                    rhs=ws[a][:, :],
                                    start=(a == 0),
                                    stop=(not with_bias) and (a == nd - 1),
                                )
                            if with_bias:
                                nc.tensor.matmul(
                                    ps[:, :], lhsT=ones[:, :], rhs=bes[:, :],
                                    start=False, stop=True,
                                )
                            ast = ap_.tile([PT, FT], F32, tag="ast", name=f"ast{h}_{ft}_{bt}")
                            nc.scalar.activation(
                                ast[:, :], ps[:, :], mybir.ActivationFunctionType.Relu
                            )
                            for g in range(FT // GRP):
                                c0 = (ft * (FT // GRP) + g) * 8
                                nc.vector.max(
                                    candV[bt][:, c0 : c0 + 8],
                                    ast[:, g * GRP : (g + 1) * GRP],
                                )
                                nc.vector.max_index(
                                    candR[bt][:, c0 : c0 + 8],
                                    candV[bt][:, c0 : c0 + 8],
                                    ast[:, g * GRP : (g + 1) * GRP],
                                )
                        if interleave is not None:
                            for _ in range(4):
                                next(interleave, None)

                def extract_half(h, candV, candR):
                    """Emit top-64 extraction for half h (all on DVE), eagerly
                    — before the next half's encode so the in-order DVE queue
                    never waits on ops behind it. Returns per-tile (t64, feat)."""
                    bts = list(range(h * nbh, (h + 1) * nbh))
                    res = {}
                    for bt in bts:
                        candPu = candV[bt][:, :].bitcast(U32)
                        nc.vector.tensor_scalar(
                            candPu, candPu, VMASK, None, mybir.AluOpType.bitwise_and
                        )
                        cr32 = tpf.tile([PT, ncand], U32, tag="cr32", name=f"cr32_{bt}", bufs=1)
                        nc.vector.tensor_copy(cr32[:, :], candR[bt][:, :])
                        nc.vector.tensor_tensor(
                            candPu, candPu, cr32[:, :], mybir.AluOpType.bitwise_or
                        )
                        t64 = tpf.tile([PT, K], F32, tag="t64", name=f"t64_{bt}", bufs=nbt)
                        pos = tpf.tile([PT, K], U32, tag="pos", name=f"pos{bt}", bufs=2)
                        for r in range(nrounds):
                            t8 = t64[:, r * 8 : (r + 1) * 8]
                            nc.vector.max(t8, candV[bt][:, :])
                            nc.vector.max_index(
                                pos[:, r * 8 : (r + 1) * 8], t8, candV[bt][:, :]
                            )
                            if r < nrounds - 1:
                                nc.vector.match_replace(
                                    candV[bt][:, :], t8, candV[bt][:, :], -1e30
                                )
                        # feat = ((pos & ~7) << 5) | (bits(t64) & 0xFF)
                        feat = tpf.tile([PT, K], U32, tag="feat", name=f"feat{bt}", bufs=nbt)
                        nc.vector.tensor_scalar(
                            feat[:, :], pos[:, :], 0xFFFFFFF8, GBITS - 3,
                            mybir.AluOpType.bitwise_and,
                            mybir.AluOpType.logical_shift_left,
                        )
                        lowt = tpf.tile([PT, K], U32, tag="lowt", name=f"lowt{bt}", bufs=2)
                        nc.vector.tensor_scalar(
                            lowt[:, :], t64[:, :].bitcast(U32), LMASK, None,
                            mybir.AluOpType.bitwise_and,
                        )
                        nc.vector.tensor_tensor(
                            feat[:, :], feat[:, :], lowt[:, :],
                            mybir.AluOpType.bitwise_or,
                        )
                        res[bt] = (t64, feat)
                    return res

                def decode_half(h, extracted):
                    """Generator emitting the sparse gather decode for half h,
                    yielding after each gather/accumulate slot. Pool generates
                    gather descriptors; 3 of 4 slots go Act-scale(fp16) +
                    DVE-2X-add, 1 of 4 goes DVE scalar_tensor_tensor solo —
                    balancing Act (~2.2us/op) against DVE (~1.35/2.6us/op)."""
                    bts = list(range(h * nbh, (h + 1) * nbh))
                    for bt in bts:
                        t64, feat = extracted[bt]
                        acc16 = accp.tile([PT, d], F16, tag="acc16", name=f"acc16_{bt}")
                        for k in range(K):
                            G = gp.tile([PT, d], BF16, tag="g", name=f"g{bt}_{k}")
                            nc.gpsimd.indirect_dma_start(
                                out=G[:, :],
                                out_offset=None,
                                in_=wdec16[:, :],
                                in_offset=bass.IndirectOffsetOnAxis(
                                    ap=feat[:, k : k + 1], axis=0
                                ),
                            )
                            val = t64[:, k : k + 1]
                            if k == 0:
                                nc.scalar.activation(
                                    acc16[:, :], G[:, :],
                                    mybir.ActivationFunctionType.Identity,
                                    scale=val,
                                )
                            elif k % 4 == 3:
                                nc.vector.scalar_tensor_tensor(
                                    acc16[:, :], G[:, :], val, acc16[:, :],
                                    mybir.AluOpType.mult, mybir.AluOpType.add,
                                )
                            else:
                                sg = sgp.tile([PT, d], F16, tag="sg", name=f"sg{bt}_{k}")
                                nc.scalar.activation(
                                    sg[:, :], G[:, :],
                                    mybir.ActivationFunctionType.Identity,
                                    scale=val,
                                )
                                nc.vector.tensor_tensor(
                                    acc16[:, :], acc16[:, :], sg[:, :],
                                    mybir.AluOpType.add,
                                )
                            yield
                        acc32 = accp.tile([PT, d], F32, tag="acc32", name=f"acc32_{bt}", bufs=1)
                        if with_bdec:
                            nc.vector.tensor_tensor(
                                acc32[:, :], acc16[:, :], bdecb_sb[:, :],
                                mybir.AluOpType.add,
                            )
                        else:
                            nc.scalar.activation(
                                acc32[:, :], acc16[:, :],
                                mybir.ActivationFunctionType.Identity,
                            )
                        nc.sync.dma_start(out=out_r[bt, :, :], in_=acc32[:, :])
                        yield

                prev_extracted = None
                for h in range(N_HALF):
                    bts = list(range(h * nbh, (h + 1) * nbh))
                    candV = {bt: cvp.tile([PT, ncand], F32, tag="cV", name=f"candV{bt}")
                             for bt in bts}
                    candR = {bt: crp.tile([PT, ncand], U16, tag="cR", name=f"candR{bt}")
                             for bt in bts}
                    prev = None
                    if prev_extracted is not None:
                        prev = decode_half(h - 1, prev_extracted)
                    encode_half(h, candV, candR, interleave=prev)
                    if prev is not None:
                        for _ in prev:
                            pass
                    prev_extracted = extract_half(h, candV, candR)
                # final half's decode
                for _ in decode_half(N_HALF - 1, prev_extracted):
                    pass

    split_waits(nc)
    return nc


def kernel(x, W_enc, b_enc, W_dec, b_dec):
    b, d = x.shape
    f = W_enc.shape[0]
    b_loc = b // N_CORES

    xT = np.ascontiguousarray(np.asarray(x, dtype=np.float32).T)       # [d, b]
    wenc = np.asarray(W_enc, dtype=np.float32)
    wencT = np.ascontiguousarray(wenc.T)                               # [d, f]
    wdec16 = np.ascontiguousarray(
        np.asarray(W_dec, dtype=np.float32).T.astype(ml_dtypes.bfloat16)
    )  # [f, d] bf16
    bdec = np.asarray(b_dec, dtype=np.float32)
    # fold the x - b_dec subtraction into the encoder bias
    benc_eff = np.asarray(b_enc, dtype=np.float32) - wenc @ bdec
    with_bias = bool(np.any(benc_eff))
    with_bdec = bool(np.any(bdec))

    nc = build_nc(b_loc, d, f, with_bias, with_bdec)

    in_maps = []
    for i in range(N_CORES):
        m = {
            "xT": np.ascontiguousarray(xT[:, i * b_loc : (i + 1) * b_loc]),
            "W_encT": wencT,
            "Wdec16": wdec16,
        }
        if with_bias:
            m["b_enc"] = benc_eff
        if with_bdec:
            m["bdecb"] = np.ascontiguousarray(np.broadcast_to(bdec, (PT, d)))
        in_maps.append(m)

    trace = bool(os.environ.get("BASS_TOPK_TRACE"))
    res = run_bass_kernel_spmd(nc, in_maps, list(range(N_CORES)), trace=trace)
    if trace and res.exec_time_ns is not None:
        print(f"HW exec time: {res.exec_time_ns} ns")
        if res.instructions_and_trace is not None:
            print(f"trace path: {res.instructions_and_trace[1]}")
        if res.profile_json is not None:
            print(f"profile json: {res.profile_json}")
    shards = [res.results[i]["out"] for i in range(N_CORES)]     # [b_loc, d] each
    return np.ascontiguousarray(np.concatenate(shards, axis=0))


if __name__ == "__main__":
    # small smoke config vs numpy simulation of the same math
    b_loc, d, f = 256, 256, 8192
    rng = np.random.default_rng(0)
    x = rng.standard_normal((N_CORES * b_loc, d), dtype=np.float32)
    W_enc = (rng.standard_normal((f, d), dtype=np.float32) / np.sqrt(d)).astype(np.float32)
    W_dec = rng.standard_normal((d, f), dtype=np.float32).astype(np.float32)

    import sys
    if "zeros" in sys.argv[1:]:
        b_enc_ = np.zeros(f, dtype=np.float32)
        b_dec_ = np.zeros(d, dtype=np.float32)
    else:
        b_enc_ = rng.standard_normal(f, dtype=np.float32) * 0.01
        b_dec_ = rng.standard_normal(d, dtype=np.float32) * 0.01

    got = kernel(x, W_enc, b_enc_, W_dec, b_dec_)

    pre = (x - b_dec_) @ W_enc.T + b_enc_
    acts = np.maximum(pre, 0)
    # simulate the kernel's group-candidate top-64 (with packed low bits)
    g = acts.reshape(acts.shape[0], -1, GRP)
    order = np.argsort(-g, axis=2, kind="stable")[:, :, :8]
    top8v = np.take_along_axis(g, order, axis=2)
    packed = ((top8v.view(np.uint32) & np.uint32(VMASK)) | order.astype(np.uint32)).view(np.float32)
    flat = packed.reshape(acts.shape[0], -1)
    srt = np.argsort(-flat, axis=1, kind="stable")[:, :K]
    vals = np.take_along_axis(flat, srt, axis=1)
    feats = ((srt & ~7) << (GBITS - 3)) | (vals.view(np.uint32) & np.uint32(LMASK))
    wd16 = W_dec.T.astype(ml_dtypes.bfloat16).astype(np.float32)  # [f, d]
    want = np.einsum("bk,bkd->bd", vals, wd16[feats]) + b_dec_
    err = np.linalg.norm(got - want) / np.linalg.norm(want)
    print("smoke rel err:", err)


# revision 33
# speedup vs baseline: 1.1260x; 1.1260x over previous
"""TopK sparse autoencoder forward pass on 8 TRN2 NeuronCores.

Strategy: data-parallel over the token batch (8192 rows -> 1024 rows/core,
zero collectives). Per core, the batch is processed in two halves so the
second half's encode overlaps the first half's sparse decode:
  1. encode: pre = x @ W_enc.T + b_eff as fp32r matmuls at full PE rate
     (batch rows on partitions, features on the free dim); the x - b_dec
     subtraction is folded into b_eff host-side. ReLU fused into the
     PSUM->SBUF copy on the Activation engine.
  2. top-64 with indices: per 256-wide feature group, DVE Max8 + MaxIndex
     give the top-8 values and their within-group positions; the position is
     packed into the low 8 mantissa bits of the value (2^-15 relative
     perturbation, far below fp32r noise). 8 rounds of Max8 + MaxIndex +
     MatchReplace over the 1152 packed candidates yield the top-64 packed
     values and their candidate positions; the feature index is recovered
     arithmetically: feat = 256*(pos>>3) | (bits(val) & 0xFF).
  3. sparse decode: for each of the 64 slots, an indirect DMA gathers
     W_dec^T rows (bf16) by feature index, one row per token partition.
     Half A (overlapped with half B's encode): Act scales G*val -> fp16,
     Pool accumulates — DVE stays free for half B's candidate scan, and
     the ops are emission-interleaved into the encode loop to avoid
     head-of-line blocking on the in-order engine queues. The final half
     accumulates with DVE scalar_tensor_tensor (Pool is busy generating
     gather descriptors).
"""

import os
import numpy as np
import ml_dtypes

from concourse import bass, mybir
from concourse import tile
from concourse.bass_utils import run_bass_kernel_spmd

F32 = mybir.dt.float32
F32R = mybir.dt.float32r
F16 = mybir.dt.float16
U32 = mybir.dt.uint32
U16 = mybir.dt.uint16
BF16 = mybir.dt.bfloat16

N_CORES = 8
B, D, F, K = 8192, 2304, 36864, 64

PT = 128           # partition tile
FT = 512           # encode feature tile (matmul moving dim)
GRP = 512          # max8 candidate group width (9-bit in-group position;
                   # P(a group holds >8 of a token's top-64) ~ 3e-5/token)
GBITS = (GRP - 1).bit_length()          # bits for the in-group position
VMASK = (~((1 << GBITS) - 1)) & 0xFFFFFFFF
LMASK = (1 << GBITS) - 1
N_HALF = 1         # batch halves for encode/decode overlap (1 = no split:
                   # re-streaming W_enc for a second half makes that half's
                   # encode DMA-bound and costs more than the overlap saves)


def split_waits(nc, maxw=1):
    """Walrus in this container accepts few sync-waits per instruction; Tile
    emits many. Move excess waits onto standalone same-engine no-ops."""
    for fn in nc.m.functions:
        for blk in fn.blocks:
            newinsts = []
            for inst in blk.instructions:
                si = inst.sync_info
                if si is not None and len(si.on_wait) > maxw:
                    extra = si.on_wait[:-maxw]
                    keep = si.on_wait[-maxw:]
                    for j, w in enumerate(extra):
                        nop = mybir.InstNoOp(name=f"{inst.name}-wsplit{j}", ins=[], outs=[])
                        nop.engine = inst.engine
                        nop.sync_info = mybir.SyncInfo(on_wait=[w], on_update=[])
                        newinsts.append(nop)
                    si.on_wait = keep
                newinsts.append(inst)
            blk.instructions = newinsts


def build_nc(b_loc, d, f, with_bias, with_bdec):
    nbt = b_loc // PT          # batch tiles
    nd = d // PT               # contraction chunks
    nft = f // FT              # encode feature tiles
    ngrp = f // GRP            # candidate groups per row
    ncand = ngrp * 8           # candidates per row
    assert ncand >= K and K % 8 == 0
    nrounds = K // 8
    assert nbt % N_HALF == 0
    nbh = nbt // N_HALF        # batch tiles per half

    nc = bass.Bass()
    # x and weights declared float32r: same 4-byte container as f32, PE
    # rounds internally — lets plain DMAs satisfy the BIR fp32r-producer
    # rule with zero cast instructions.
    xT = nc.declare_dram_parameter("xT", [d, b_loc], F32R, isOutput=False)
    wencT = nc.declare_dram_parameter("W_encT", [d, f], F32R, isOutput=False)
    wdec16 = nc.declare_dram_parameter("Wdec16", [f, d], BF16, isOutput=False)
    if with_bias:
        b_enc = nc.declare_dram_parameter("b_enc", [f], F32R, isOutput=False)
    if with_bdec:
        bdecb = nc.declare_dram_parameter("bdecb", [PT, d], F32, isOutput=False)
    out = nc.declare_dram_parameter("out", [b_loc, d], F32, isOutput=True)

    wencT_r = wencT.rearrange("(a p) f -> p a f", p=PT)   # [128, nd, f]
    xT_r = xT.rearrange("(a p) b -> p a b", p=PT)         # [128, nd, b_loc]
    out_r = out.rearrange("(t p) e -> t p e", p=PT)       # [nbt, 128, d]

    with tile.TileContext(nc) as tc:
        with tc.tile_pool(name="persist", bufs=1) as pp:
            if with_bias:
                ones_st = pp.tile([1, PT], F32)
                nc.vector.memset(ones_st[:, :], 1.0)
                ones = pp.tile([1, PT], F32R)
                nc.vector.tensor_copy(ones[:, :], ones_st[:, :])
            if with_bdec:
                bdecb_sb = pp.tile([PT, d], F32)
                nc.sync.dma_start(out=bdecb_sb[:, :], in_=bdecb[:, :])

            # x tiles, resident for the whole encode
            xs = []
            for a in range(nd):
                xt = pp.tile([PT, b_loc], F32R, name=f"xs{a}")
                nc.sync.dma_start(out=xt[:, :], in_=xT_r[:, a, :])
                xs.append(xt)

            with tc.tile_pool(name="tpf", bufs=8) as tpf:

                def encode_half(h, candV, candR, interleave=None):
                    """Emit encode+candidate ops for half h; pull from the
                    `interleave` generator (previous half's decode) after
                    each feature tile to keep the in-order engine queues
                    from head-blocking."""
                    bts = list(range(h * nbh, (h + 1) * nbh))
                    for ft in range(nft):
                        f0 = ft * FT
                        ws = []
                        for a in range(nd):
                            wst = wp.tile([PT, FT], F32R, tag="wst", name=f"wst{h}_{ft}_{a}")
                            nc.sync.dma_start(
                                out=wst[:, :], in_=wencT_r[:, a, f0 : f0 + FT]
                            )
                            ws.append(wst)
                        if with_bias:
                            bes = bp.tile([1, FT], F32R, tag="bes", name=f"bes{h}_{ft}")
                            nc.sync.dma_start(
                                out=bes[:, :],
                                in_=b_enc.rearrange("(o x) -> o x", o=1)[:, f0 : f0 + FT],
                            )
                        for bt in bts:
                            ps = pse.tile([PT, FT], F32, tag="pse", name=f"pse{h}_{ft}_{bt}")
                            for a in range(nd):
                                nc.tensor.matmul(
                                    ps[:, :],
                                    lhsT=xs[a][:, bt * PT : (bt + 1) * PT],
                                    rhs=ws[a][:, :],
                                    start=(a == 0),
                                    stop=(not with_bias) and (a == nd - 1),
                                )
                            if with_bias:
                                nc.tensor.matmul(
                                    ps[:, :], lhsT=ones[:, :], rhs=bes[:, :],
                                    start=False, stop=True,
                                )
                            ast = ap_.tile([PT, FT], F32, tag="ast", name=f"ast{h}_{ft}_{bt}")
                            nc.scalar.activation(
                                ast[:, :], ps[:, :], mybir.ActivationFunctionType.Relu
                            )
                            for g in range(FT // GRP):
                                c0 = (ft * (FT // GRP) + g) * 8
                                nc.vector.max(
                                    candV[bt][:, c0 : c0 + 8],
                                    ast[:, g * GRP : (g + 1) * GRP],
                                )
                                nc.vector.max_index(
                                    candR[bt][:, c0 : c0 + 8],
                                    candV[bt][:, c0 : c0 + 8],
                                    ast[:, g * GRP : (g + 1) * GRP],
                                )
                        if interleave is not None:
                            for _ in range(4):
                                next(interleave, None)

                def extract_half(h, candV, candR):
                    """Emit top-64 extraction for half h (all on DVE), eagerly
                    — before the next half's encode so the in-order DVE queue
                    never waits on ops behind it. Returns per-tile (t64, feat)."""
                    bts = list(range(h * nbh, (h + 1) * nbh))
                    res = {}
                    for bt in bts:
                        candPu = candV[bt][:, :].bitcast(U32)
                        nc.vector.tensor_scalar(
                            candPu, candPu, VMASK, None, mybir.AluOpType.bitwise_and
                        )
                        cr32 = tpf.tile([PT, ncand], U32, tag="cr32", name=f"cr32_{bt}", bufs=1)
                        nc.vector.tensor_copy(cr32[:, :], candR[bt][:, :])
                        nc.vector.tensor_tensor(
                            candPu, candPu, cr32[:, :], mybir.AluOpType.bitwise_or
                        )
                        t64 = tpf.tile([PT, K], F32, tag="t64", name=f"t64_{bt}", bufs=nbt)
                        pos = tpf.tile([PT, K], U32, tag="pos", name=f"pos{bt}", bufs=2)
                        for r in range(nrounds):
                            t8 = t64[:, r * 8 : (r + 1) * 8]
                            nc.vector.max(t8, candV[bt][:, :])
                            nc.vector.max_index(
                                pos[:, r * 8 : (r + 1) * 8], t8, candV[bt][:, :]
                            )
                            if r < nrounds - 1:
                                nc.vector.match_replace(
                                    candV[bt][:, :], t8, candV[bt][:, :], -1e30
                                )
                        # feat = ((pos & ~7) << 5) | (bits(t64) & 0xFF)
                        feat = tpf.tile([PT, K], U32, tag="feat", name=f"feat{bt}", bufs=nbt)
                        nc.vector.tensor_scalar(
                            feat[:, :], pos[:, :], 0xFFFFFFF8, GBITS - 3,
                            mybir.AluOpType.bitwise_and,
                            mybir.AluOpType.logical_shift_left,
                        )
                        lowt = tpf.tile([PT, K], U32, tag="lowt", name=f"lowt{bt}", bufs=2)
                        nc.vector.tensor_scalar(
                            lowt[:, :], t64[:, :].bitcast(U32), LMASK, None,
                            mybir.AluOpType.bitwise_and,
                        )
                        nc.vector.tensor_tensor(
                            feat[:, :], feat[:, :], lowt[:, :],
                            mybir.AluOpType.bitwise_or,
                        )
                        res[bt] = (t64, feat)
                    return res

                GROUP = 4  # tiles whose decode slots interleave round-robin

                def decode_half(h, extracted):
                    """Generator emitting the sparse gather decode for half h.
                    Slots are emitted round-robin across GROUP tiles so the
                    in-order engine queues never head-block on one tile's
                    serial accumulate chain. Pool generates gather
                    descriptors; 3 of 4 slots go Act-scale(fp16) + DVE-2X-add,
                    1 of 4 goes DVE scalar_tensor_tensor solo."""
                    bts = list(range(h * nbh, (h + 1) * nbh))
                    for g0 in range(0, len(bts), GROUP):
                        grp = bts[g0 : g0 + GROUP]
                        acc16s = {bt: accp.tile([PT, d], F16, tag="acc16", name=f"acc16_{bt}")
                                  for bt in grp}
                        for k in range(K):
                            for bt in grp:
                                t64, feat = extracted[bt]
                                acc16 = acc16s[bt]
                                G = gp.tile([PT, d], BF16, tag="g", name=f"g{bt}_{k}")
                                nc.gpsimd.indirect_dma_start(
                                    out=G[:, :],
                                    out_offset=None,
                                    in_=wdec16[:, :],
                                    in_offset=bass.IndirectOffsetOnAxis(
                                        ap=feat[:, k : k + 1], axis=0
                                    ),
                                )
                                val = t64[:, k : k + 1]
                                if k == 0:
                                    nc.scalar.activation(
                                        acc16[:, :], G[:, :],
                                        mybir.ActivationFunctionType.Identity,
                                        scale=val,
                                    )
                                elif k % 4 == 3:
                                    nc.vector.scalar_tensor_tensor(
                                        acc16[:, :], G[:, :], val, acc16[:, :],
                                        mybir.AluOpType.mult, mybir.AluOpType.add,
                                    )
                                else:
                                    sg = sgp.tile([PT, d], F16, tag="sg", name=f"sg{bt}_{k}")
                                    nc.scalar.activation(
                                        sg[:, :], G[:, :],
                                        mybir.ActivationFunctionType.Identity,
                                        scale=val,
                                    )
                                    nc.vector.tensor_tensor(
                                        acc16[:, :], acc16[:, :], sg[:, :],
                                        mybir.AluOpType.add,
                                    )
                                yield
                        for bt in grp:
                            acc16 = acc16s[bt]
                            acc32 = accp.tile([PT, d], F32, tag="acc32", name=f"acc32_{bt}", bufs=2)
                            if with_bdec:
                                nc.vector.tensor_tensor(
                                    acc32[:, :], acc16[:, :], bdecb_sb[:, :],
                                    mybir.AluOpType.add,
                                )
                            else:
                                nc.scalar.activation(
                                    acc32[:, :], acc16[:, :],
                                    mybir.ActivationFunctionType.Identity,
                                )
                            nc.sync.dma_start(out=out_r[bt, :, :], in_=acc32[:, :])
                            yield

                assert N_HALF == 1
                with tc.tile_pool(name="candV", bufs=nbh) as cvp, \
                     tc.tile_pool(name="candR", bufs=nbh) as crp, \
                     tc.tile_pool(name="enc_w", bufs=nd + 6) as wp, \
                     tc.tile_pool(name="enc_b", bufs=2) as bp, \
                     tc.tile_pool(name="enc_ast", bufs=4) as ap_, \
                     tc.tile_pool(name="psum_e", bufs=4, space="PSUM") as pse:
                    candV = {bt: cvp.tile([PT, ncand], F32, tag="cV", name=f"candV{bt}")
                             for bt in range(nbt)}
                    candR = {bt: crp.tile([PT, ncand], U16, tag="cR", name=f"candR{bt}")
                             for bt in range(nbt)}
                    encode_half(0, candV, candR)
                    extracted = extract_half(0, candV, candR)
                with tc.tile_pool(name="dec_g", bufs=12) as gp, \
                     tc.tile_pool(name="dec_sg", bufs=8) as sgp, \
                     tc.tile_pool(name="dec_acc", bufs=4) as accp:
                    for _ in decode_half(0, extracted):
                        pass

    split_waits(nc)
    return nc


def kernel(x, W_enc, b_enc, W_dec, b_dec):
    b, d = x.shape
    f = W_enc.shape[0]
    b_loc = b // N_CORES

    xT = np.ascontiguousarray(np.asarray(x, dtype=np.float32).T)       # [d, b]
    wenc = np.asarray(W_enc, dtype=np.float32)
    wencT = np.ascontiguousarray(wenc.T)                               # [d, f]
    wdec16 = np.ascontiguousarray(
        np.asarray(W_dec, dtype=np.float32).T.astype(ml_dtypes.bfloat16)
    )  # [f, d] bf16
    bdec = np.asarray(b_dec, dtype=np.float32)
    # fold the x - b_dec subtraction into the encoder bias
    benc_eff = np.asarray(b_enc, dtype=np.float32) - wenc @ bdec
    with_bias = bool(np.any(benc_eff))
    with_bdec = bool(np.any(bdec))

    nc = build_nc(b_loc, d, f, with_bias, with_bdec)

    in_maps = []
    for i in range(N_CORES):
        m = {
            "xT": np.ascontiguousarray(xT[:, i * b_loc : (i + 1) * b_loc]),
            "W_encT": wencT,
            "Wdec16": wdec16,
        }
        if with_bias:
            m["b_enc"] = benc_eff
        if with_bdec:
            m["bdecb"] = np.ascontiguousarray(np.broadcast_to(bdec, (PT, d)))
        in_maps.append(m)

    trace = bool(os.environ.get("BASS_TOPK_TRACE"))
    res = run_bass_kernel_spmd(nc, in_maps, list(range(N_CORES)), trace=trace)
    if trace and res.exec_time_ns is not None:
        print(f"HW exec time: {res.exec_time_ns} ns")
        if res.instructions_and_trace is not None:
            print(f"trace path: {res.instructions_and_trace[1]}")
        if res.profile_json is not None:
            print(f"profile json: {res.profile_json}")
    shards = [res.results[i]["out"] for i in range(N_CORES)]     # [b_loc, d] each
    return np.ascontiguousarray(np.concatenate(shards, axis=0))


if __name__ == "__main__":
    # small smoke config vs numpy simulation of the same math
    b_loc, d, f = 256, 256, 8192
    rng = np.random.default_rng(0)
    x = rng.standard_normal((N_CORES * b_loc, d), dtype=np.float32)
    W_enc = (rng.standard_normal((f, d), dtype=np.float32) / np.sqrt(d)).astype(np.float32)
    W_dec = rng.standard_normal((d, f), dtype=np.float32).astype(np.float32)

    import sys
    if "zeros" in sys.argv[1:]:
        b_enc_ = np.zeros(f, dtype=np.float32)
        b_dec_ = np.zeros(d, dtype=np.float32)
    else:
        b_enc_ = rng.standard_normal(f, dtype=np.float32) * 0.01
        b_dec_ = rng.standard_normal(d, dtype=np.float32) * 0.01

    got = kernel(x, W_enc, b_enc_, W_dec, b_dec_)

    pre = (x - b_dec_) @ W_enc.T + b_enc_
    acts = np.maximum(pre, 0)
    # simulate the kernel's group-candidate top-64 (with packed low bits)
    g = acts.reshape(acts.shape[0], -1, GRP)
    order = np.argsort(-g, axis=2, kind="stable")[:, :, :8]
    top8v = np.take_along_axis(g, order, axis=2)
    packed = ((top8v.view(np.uint32) & np.uint32(VMASK)) | order.astype(np.uint32)).view(np.float32)
    flat = packed.reshape(acts.shape[0], -1)
    srt = np.argsort(-flat, axis=1, kind="stable")[:, :K]
    vals = np.take_along_axis(flat, srt, axis=1)
    feats = ((srt & ~7) << (GBITS - 3)) | (vals.view(np.uint32) & np.uint32(LMASK))
    wd16 = W_dec.T.astype(ml_dtypes.bfloat16).astype(np.float32)  # [f, d]
    want = np.einsum("bk,bkd->bd", vals, wd16[feats]) + b_dec_
    err = np.linalg.norm(got - want) / np.linalg.norm(want)
    print("smoke rel err:", err)


# revision 34
# speedup vs baseline: 1.1381x; 1.0107x over previous
"""TopK sparse autoencoder forward pass on 8 TRN2 NeuronCores.

Strategy: data-parallel over the token batch (8192 rows -> 1024 rows/core,
zero collectives). Per core, the batch is processed in two halves so the
second half's encode overlaps the first half's sparse decode:
  1. encode: pre = x @ W_enc.T + b_eff as fp32r matmuls at full PE rate
     (batch rows on partitions, features on the free dim); the x - b_dec
     subtraction is folded into b_eff host-side. ReLU fused into the
     PSUM->SBUF copy on the Activation engine.
  2. top-64 with indices: per 256-wide feature group, DVE Max8 + MaxIndex
     give the top-8 values and their within-group positions; the position is
     packed into the low 8 mantissa bits of the value (2^-15 relative
     perturbation, far below fp32r noise). 8 rounds of Max8 + MaxIndex +
     MatchReplace over the 1152 packed candidates yield the top-64 packed
     values and their candidate positions; the feature index is recovered
     arithmetically: feat = 256*(pos>>3) | (bits(val) & 0xFF).
  3. sparse decode: for each of the 64 slots, an indirect DMA gathers
     W_dec^T rows (bf16) by feature index, one row per token partition.
     Half A (overlapped with half B's encode): Act scales G*val -> fp16,
     Pool accumulates — DVE stays free for half B's candidate scan, and
     the ops are emission-interleaved into the encode loop to avoid
     head-of-line blocking on the in-order engine queues. The final half
     accumulates with DVE scalar_tensor_tensor (Pool is busy generating
     gather descriptors).
"""

import os
import numpy as np
import ml_dtypes

from concourse import bass, mybir
from concourse import tile
import concourse.bass_utils as _bu
from concourse.bass_utils import run_bass_kernel_spmd

if os.environ.get("BASS_LDW_OPT"):
    _orig_run_command = _bu.run_command

    def _run_command_ldw(argv, **kw):
        argv = ["--enable-ldw-opt=true" if a == "--enable-ldw-opt=false" else a
                for a in argv]
        return _orig_run_command(argv, **kw)

    _bu.run_command = _run_command_ldw

F32 = mybir.dt.float32
F32R = mybir.dt.float32r
F16 = mybir.dt.float16
U32 = mybir.dt.uint32
U16 = mybir.dt.uint16
BF16 = mybir.dt.bfloat16

N_CORES = 8
B, D, F, K = 8192, 2304, 36864, 64

PT = 128           # partition tile
FT = 512           # encode feature tile (matmul moving dim)
GRP = 512          # max8 candidate group width (9-bit in-group position;
                   # P(a group holds >8 of a token's top-64) ~ 3e-5/token)
GBITS = (GRP - 1).bit_length()          # bits for the in-group position
VMASK = (~((1 << GBITS) - 1)) & 0xFFFFFFFF
LMASK = (1 << GBITS) - 1
N_HALF = 1         # batch halves for encode/decode overlap (1 = no split:
                   # re-streaming W_enc for a second half makes that half's
                   # encode DMA-bound and costs more than the overlap saves)


def split_waits(nc, maxw=1):
    """Walrus in this container accepts few sync-waits per instruction; Tile
    emits many. Move excess waits onto standalone same-engine no-ops."""
    for fn in nc.m.functions:
        for blk in fn.blocks:
            newinsts = []
            for inst in blk.instructions:
                si = inst.sync_info
                if si is not None and len(si.on_wait) > maxw:
                    extra = si.on_wait[:-maxw]
                    keep = si.on_wait[-maxw:]
                    for j, w in enumerate(extra):
                        nop = mybir.InstNoOp(name=f"{inst.name}-wsplit{j}", ins=[], outs=[])
                        nop.engine = inst.engine
                        nop.sync_info = mybir.SyncInfo(on_wait=[w], on_update=[])
                        newinsts.append(nop)
                    si.on_wait = keep
                newinsts.append(inst)
            blk.instructions = newinsts


def build_nc(b_loc, d, f, with_bias, with_bdec):
    nbt = b_loc // PT          # batch tiles
    nd = d // PT               # contraction chunks
    nft = f // FT              # encode feature tiles
    ngrp = f // GRP            # candidate groups per row
    ncand = ngrp * 8           # candidates per row
    assert ncand >= K and K % 8 == 0
    nrounds = K // 8
    assert nbt % N_HALF == 0
    nbh = nbt // N_HALF        # batch tiles per half

    nc = bass.Bass()
    # x and weights declared float32r: same 4-byte container as f32, PE
    # rounds internally — lets plain DMAs satisfy the BIR fp32r-producer
    # rule with zero cast instructions.
    xT = nc.declare_dram_parameter("xT", [d, b_loc], F32R, isOutput=False)
    wencT = nc.declare_dram_parameter("W_encT", [d, f], F32R, isOutput=False)
    wdec16 = nc.declare_dram_parameter("Wdec16", [f, d], BF16, isOutput=False)
    if with_bias:
        b_enc = nc.declare_dram_parameter("b_enc", [f], F32R, isOutput=False)
    if with_bdec:
        bdecb = nc.declare_dram_parameter("bdecb", [PT, d], F32, isOutput=False)
    out = nc.declare_dram_parameter("out", [b_loc, d], F32, isOutput=True)

    wencT_r = wencT.rearrange("(a p) f -> p a f", p=PT)   # [128, nd, f]
    xT_r = xT.rearrange("(a p) b -> p a b", p=PT)         # [128, nd, b_loc]
    out_r = out.rearrange("(t p) e -> t p e", p=PT)       # [nbt, 128, d]

    with tile.TileContext(nc) as tc:
        with tc.tile_pool(name="persist", bufs=1) as pp:
            if with_bias:
                ones_st = pp.tile([1, PT], F32)
                nc.vector.memset(ones_st[:, :], 1.0)
                ones = pp.tile([1, PT], F32R)
                nc.vector.tensor_copy(ones[:, :], ones_st[:, :])
            if with_bdec:
                bdecb_sb = pp.tile([PT, d], F32)
                nc.sync.dma_start(out=bdecb_sb[:, :], in_=bdecb[:, :])

            # x tiles, resident for the whole encode
            xs = []
            for a in range(nd):
                xt = pp.tile([PT, b_loc], F32R, name=f"xs{a}")
                nc.sync.dma_start(out=xt[:, :], in_=xT_r[:, a, :])
                xs.append(xt)

            with tc.tile_pool(name="tpf", bufs=8) as tpf:

                def encode_half(h, candV, candR, interleave=None):
                    """Emit encode+candidate ops for half h; pull from the
                    `interleave` generator (previous half's decode) after
                    each feature tile to keep the in-order engine queues
                    from head-blocking."""
                    bts = list(range(h * nbh, (h + 1) * nbh))
                    for ft in range(nft):
                        f0 = ft * FT
                        ws = []
                        for a in range(nd):
                            wst = wp.tile([PT, FT], F32R, tag="wst", name=f"wst{h}_{ft}_{a}")
                            nc.sync.dma_start(
                                out=wst[:, :], in_=wencT_r[:, a, f0 : f0 + FT]
                            )
                            ws.append(wst)
                        if with_bias:
                            bes = bp.tile([1, FT], F32R, tag="bes", name=f"bes{h}_{ft}")
                            nc.sync.dma_start(
                                out=bes[:, :],
                                in_=b_enc.rearrange("(o x) -> o x", o=1)[:, f0 : f0 + FT],
                            )
                        for bt in bts:
                            ps = pse.tile([PT, FT], F32, tag="pse", name=f"pse{h}_{ft}_{bt}")
                            for a in range(nd):
                                nc.tensor.matmul(
                                    ps[:, :],
                                    lhsT=xs[a][:, bt * PT : (bt + 1) * PT],
                                    rhs=ws[a][:, :],
                                    start=(a == 0),
                                    stop=(not with_bias) and (a == nd - 1),
                                )
                            if with_bias:
                                nc.tensor.matmul(
                                    ps[:, :], lhsT=ones[:, :], rhs=bes[:, :],
                                    start=False, stop=True,
                                )
                            ast = ap_.tile([PT, FT], F32, tag="ast", name=f"ast{h}_{ft}_{bt}")
                            nc.scalar.activation(
                                ast[:, :], ps[:, :], mybir.ActivationFunctionType.Relu
                            )
                            for g in range(FT // GRP):
                                c0 = (ft * (FT // GRP) + g) * 8
                                nc.vector.max(
                                    candV[bt][:, c0 : c0 + 8],
                                    ast[:, g * GRP : (g + 1) * GRP],
                                )
                                nc.vector.max_index(
                                    candR[bt][:, c0 : c0 + 8],
                                    candV[bt][:, c0 : c0 + 8],
                                    ast[:, g * GRP : (g + 1) * GRP],
                                )
                        if interleave is not None:
                            for _ in range(4):
                                next(interleave, None)

                def extract_half(h, candV, candR):
                    """Emit top-64 extraction for half h (all on DVE), eagerly
                    — before the next half's encode so the in-order DVE queue
                    never waits on ops behind it. Returns per-tile (t64, feat)."""
                    bts = list(range(h * nbh, (h + 1) * nbh))
                    res = {}
                    for bt in bts:
                        candPu = candV[bt][:, :].bitcast(U32)
                        nc.vector.tensor_scalar(
                            candPu, candPu, VMASK, None, mybir.AluOpType.bitwise_and
                        )
                        cr32 = tpf.tile([PT, ncand], U32, tag="cr32", name=f"cr32_{bt}", bufs=1)
                        nc.vector.tensor_copy(cr32[:, :], candR[bt][:, :])
                        nc.vector.tensor_tensor(
                            candPu, candPu, cr32[:, :], mybir.AluOpType.bitwise_or
                        )
                        t64 = tpf.tile([PT, K], F32, tag="t64", name=f"t64_{bt}", bufs=nbt)
                        pos = tpf.tile([PT, K], U32, tag="pos", name=f"pos{bt}", bufs=2)
                        for r in range(nrounds):
                            t8 = t64[:, r * 8 : (r + 1) * 8]
                            nc.vector.max(t8, candV[bt][:, :])
                            nc.vector.max_index(
                                pos[:, r * 8 : (r + 1) * 8], t8, candV[bt][:, :]
                            )
                            if r < nrounds - 1:
                                nc.vector.match_replace(
                                    candV[bt][:, :], t8, candV[bt][:, :], -1e30
                                )
                        # feat = ((pos & ~7) << 5) | (bits(t64) & 0xFF)
                        feat = tpf.tile([PT, K], U32, tag="feat", name=f"feat{bt}", bufs=nbt)
                        nc.vector.tensor_scalar(
                            feat[:, :], pos[:, :], 0xFFFFFFF8, GBITS - 3,
                            mybir.AluOpType.bitwise_and,
                            mybir.AluOpType.logical_shift_left,
                        )
                        lowt = tpf.tile([PT, K], U32, tag="lowt", name=f"lowt{bt}", bufs=2)
                        nc.vector.tensor_scalar(
                            lowt[:, :], t64[:, :].bitcast(U32), LMASK, None,
                            mybir.AluOpType.bitwise_and,
                        )
                        nc.vector.tensor_tensor(
                            feat[:, :], feat[:, :], lowt[:, :],
                            mybir.AluOpType.bitwise_or,
                        )
                        res[bt] = (t64, feat)
                    return res

                GROUP = 4  # tiles whose decode slots interleave round-robin

                def decode_half(h, extracted):
                    """Generator emitting the sparse gather decode for half h.
                    Slots are emitted round-robin across GROUP tiles so the
                    in-order engine queues never head-block on one tile's
                    serial accumulate chain. Pool generates gather
                    descriptors; 3 of 4 slots go Act-scale(fp16) + DVE-2X-add,
                    1 of 4 goes DVE scalar_tensor_tensor solo."""
                    bts = list(range(h * nbh, (h + 1) * nbh))
                    for g0 in range(0, len(bts), GROUP):
                        grp = bts[g0 : g0 + GROUP]
                        acc16s = {bt: accp.tile([PT, d], F16, tag="acc16", name=f"acc16_{bt}")
                                  for bt in grp}
                        for k in range(K):
                            for bt in grp:
                                t64, feat = extracted[bt]
                                acc16 = acc16s[bt]
                                G = gp.tile([PT, d], BF16, tag="g", name=f"g{bt}_{k}")
                                nc.gpsimd.indirect_dma_start(
                                    out=G[:, :],
                                    out_offset=None,
                                    in_=wdec16[:, :],
                                    in_offset=bass.IndirectOffsetOnAxis(
                                        ap=feat[:, k : k + 1], axis=0
                                    ),
                                )
                                val = t64[:, k : k + 1]
                                if k == 0:
                                    nc.scalar.activation(
                                        acc16[:, :], G[:, :],
                                        mybir.ActivationFunctionType.Identity,
                                        scale=val,
                                    )
                                elif k % 4 == 3:
                                    nc.vector.scalar_tensor_tensor(
                                        acc16[:, :], G[:, :], val, acc16[:, :],
                                        mybir.AluOpType.mult, mybir.AluOpType.add,
                                    )
                                else:
                                    sg = sgp.tile([PT, d], F16, tag="sg", name=f"sg{bt}_{k}")
                                    nc.scalar.activation(
                                        sg[:, :], G[:, :],
                                        mybir.ActivationFunctionType.Identity,
                                        scale=val,
                                    )
                                    nc.vector.tensor_tensor(
                                        acc16[:, :], acc16[:, :], sg[:, :],
                                        mybir.AluOpType.add,
                                    )
                                yield
                        for bt in grp:
                            acc16 = acc16s[bt]
                            acc32 = accp.tile([PT, d], F32, tag="acc32", name=f"acc32_{bt}", bufs=2)
                            if with_bdec:
                                nc.vector.tensor_tensor(
                                    acc32[:, :], acc16[:, :], bdecb_sb[:, :],
                                    mybir.AluOpType.add,
                                )
                            else:
                                nc.scalar.activation(
                                    acc32[:, :], acc16[:, :],
                                    mybir.ActivationFunctionType.Identity,
                                )
                            nc.sync.dma_start(out=out_r[bt, :, :], in_=acc32[:, :])
                            yield

                assert N_HALF == 1
                with tc.tile_pool(name="candV", bufs=nbh) as cvp, \
                     tc.tile_pool(name="candR", bufs=nbh) as crp, \
                     tc.tile_pool(name="enc_w", bufs=nd + 6) as wp, \
                     tc.tile_pool(name="enc_b", bufs=2) as bp, \
                     tc.tile_pool(name="enc_ast", bufs=4) as ap_, \
                     tc.tile_pool(name="psum_e", bufs=4, space="PSUM") as pse:
                    candV = {bt: cvp.tile([PT, ncand], F32, tag="cV", name=f"candV{bt}")
                             for bt in range(nbt)}
                    candR = {bt: crp.tile([PT, ncand], U16, tag="cR", name=f"candR{bt}")
                             for bt in range(nbt)}
                    encode_half(0, candV, candR)
                    extracted = extract_half(0, candV, candR)
                with tc.tile_pool(name="dec_g", bufs=12) as gp, \
                     tc.tile_pool(name="dec_sg", bufs=8) as sgp, \
                     tc.tile_pool(name="dec_acc", bufs=4) as accp:
                    for _ in decode_half(0, extracted):
                        pass

    split_waits(nc)
    return nc


def kernel(x, W_enc, b_enc, W_dec, b_dec):
    b, d = x.shape
    f = W_enc.shape[0]
    b_loc = b // N_CORES

    xT = np.ascontiguousarray(np.asarray(x, dtype=np.float32).T)       # [d, b]
    wenc = np.asarray(W_enc, dtype=np.float32)
    wencT = np.ascontiguousarray(wenc.T)                               # [d, f]
    wdec16 = np.ascontiguousarray(
        np.asarray(W_dec, dtype=np.float32).T.astype(ml_dtypes.bfloat16)
    )  # [f, d] bf16
    bdec = np.asarray(b_dec, dtype=np.float32)
    # fold the x - b_dec subtraction into the encoder bias
    benc_eff = np.asarray(b_enc, dtype=np.float32) - wenc @ bdec
    with_bias = bool(np.any(benc_eff))
    with_bdec = bool(np.any(bdec))

    nc = build_nc(b_loc, d, f, with_bias, with_bdec)

    in_maps = []
    for i in range(N_CORES):
        m = {
            "xT": np.ascontiguousarray(xT[:, i * b_loc : (i + 1) * b_loc]),
            "W_encT": wencT,
            "Wdec16": wdec16,
        }
        if with_bias:
            m["b_enc"] = benc_eff
        if with_bdec:
            m["bdecb"] = np.ascontiguousarray(np.broadcast_to(bdec, (PT, d)))
        in_maps.append(m)

    trace = bool(os.environ.get("BASS_TOPK_TRACE"))
    res = run_bass_kernel_spmd(nc, in_maps, list(range(N_CORES)), trace=trace)
    if trace and res.exec_time_ns is not None:
        print(f"HW exec time: {res.exec_time_ns} ns")
        if res.instructions_and_trace is not None:
            print(f"trace path: {res.instructions_and_trace[1]}")
        if res.profile_json is not None:
            print(f"profile json: {res.profile_json}")
    shards = [res.results[i]["out"] for i in range(N_CORES)]     # [b_loc, d] each
    return np.ascontiguousarray(np.concatenate(shards, axis=0))


if __name__ == "__main__":
    # small smoke config vs numpy simulation of the same math
    b_loc, d, f = 256, 256, 8192
    rng = np.random.default_rng(0)
    x = rng.standard_normal((N_CORES * b_loc, d), dtype=np.float32)
    W_enc = (rng.standard_normal((f, d), dtype=np.float32) / np.sqrt(d)).astype(np.float32)
    W_dec = rng.standard_normal((d, f), dtype=np.float32).astype(np.float32)

    import sys
    if "zeros" in sys.argv[1:]:
        b_enc_ = np.zeros(f, dtype=np.float32)
        b_dec_ = np.zeros(d, dtype=np.float32)
    else:
        b_enc_ = rng.standard_normal(f, dtype=np.float32) * 0.01
        b_dec_ = rng.standard_normal(d, dtype=np.float32) * 0.01

    got = kernel(x, W_enc, b_enc_, W_dec, b_dec_)

    pre = (x - b_dec_) @ W_enc.T + b_enc_
    acts = np.maximum(pre, 0)
    # simulate the kernel's group-candidate top-64 (with packed low bits)
    g = acts.reshape(acts.shape[0], -1, GRP)
    order = np.argsort(-g, axis=2, kind="stable")[:, :, :8]
    top8v = np.take_along_axis(g, order, axis=2)
    packed = ((top8v.view(np.uint32) & np.uint32(VMASK)) | order.astype(np.uint32)).view(np.float32)
    flat = packed.reshape(acts.shape[0], -1)
    srt = np.argsort(-flat, axis=1, kind="stable")[:, :K]
    vals = np.take_along_axis(flat, srt, axis=1)
    feats = ((srt & ~7) << (GBITS - 3)) | (vals.view(np.uint32) & np.uint32(LMASK))
    wd16 = W_dec.T.astype(ml_dtypes.bfloat16).astype(np.float32)  # [f, d]
    want = np.einsum("bk,bkd->bd", vals, wd16[feats]) + b_dec_
    err = np.linalg.norm(got - want) / np.linalg.norm(want)
    print("smoke rel err:", err)
